# revision 13
# baseline (speedup 1.0000x reference)
"""Trainium2 Bass kernel for nn_MoRAPEForCausalLM (MoR expert-choice routing).

Self-contained. kernel(**inputs) -> np.ndarray [2, 2048, 32000] fp32.

Sharding (8 cores, SPMD single NEFF): tokens sharded (batch = core//4,
quarter = core%4); activations feature-major [D, T] in SBUF; K/V + routing
state exchanged via AllGather; device-side top-k (threshold bisection +
prefix-sum compaction + indirect DMA); lm_head vocab-sharded. Per-core
behavior via partition_id registers (dynamic DMA slices) + per-core small
inputs (attention-rank exp bias).

Precision: hi/lo fp32r (3-pass ~fp32) for router-feeding matmuls (blocks 0-2,
AV single-pass), fp16 blocks 3/4 (T=128), fp32r block 5 + lm_head, exact fp32
DVE router matvecs.
"""
import math

import numpy as np

import concourse.bass as bass
import concourse.mybir as mybir
import concourse.tile as tile
from concourse import bacc
from concourse.bass import ts, ds
from concourse.bass_utils import run_bass_kernel_spmd
from concourse.expressions import smax
from concourse.masks import make_identity

P = 128
f32 = mybir.dt.float32
f32r = mybir.dt.float32r
f16 = mybir.dt.float16
i32 = mybir.dt.int32
AF = mybir.ActivationFunctionType
OP = mybir.AluOpType

B, S, D, H, DH, F, V = 2, 2048, 1024, 16, 64, 4096, 32000
R, NRANK = 8, 4
ALPHA, EPS = 0.1, 1e-6
KD, KF = D // P, F // P
T0 = B * S // R          # 512
T1 = T0 // 2             # 256
T2 = T0 // 4             # 128
VS = V // R              # 4000
ISQ = 1.0 / math.sqrt(DH)

BLOCK_PREC = ('hilo', 'hilo', 'hilo', 'f16', 'f16', 'f32r')
BISECT_ITERS = 26
KGRP = 8
REPL = [list(range(R))]
NEG = -30.0

WSHAPES = {'wq': (D, D), 'wk': (D, D), 'wv': (D, D), 'wo': (D, D),
           'wg': (D, F), 'wu': (D, F), 'wd': (F, D)}
WNAMES = ('wq', 'wk', 'wv', 'wo', 'wg', 'wu', 'wd')
REFNAMES = {'wq': 'Wq', 'wk': 'Wk', 'wv': 'Wv', 'wo': 'Wo',
            'wg': 'Wg', 'wu': 'Wu', 'wd': 'Wd'}


def _round11(a):
    ai = a.view(np.uint32).astype(np.int64)
    out = ((ai + (1 << 11)) & ~((1 << 12) - 1)) & 0xFFFFFFFF
    return out.astype(np.uint32).view(np.float32)


def _dt(prec):
    return f16 if prec == 'f16' else f32r


def _passes(prec, lh, ll, rh, rl):
    if prec == 'hilo':
        return [(lh, rh), (lh, rl), (ll, rh)]
    return [(lh, rh)]


def make_pack_meta():
    meta = {}
    for blk in range(6):
        prec = BLOCK_PREC[blk]
        parts = ('h', 'l') if prec == 'hilo' else ('h',)
        items = []
        off = 0
        for wn in WNAMES:
            rows, cols = WSHAPES[wn]
            for part in parts:
                items.append((f"{wn}_{part}", rows, cols, off))
                off += (rows // R) * cols
        meta[blk] = (items, off)
    return meta


PACK_META = make_pack_meta()


class CX:
    pass


def wview(cx, blk, key, ko, c0, cn):
    items, _ = PACK_META[blk]
    for k, rows, cols, off in items:
        if k == key:
            rpr = rows // R
            row0 = ko * P
            rank, rrow = row0 // rpr, row0 % rpr
            gp = cx.wpacks[blk]
            apv = gp[rank, ds(off + rrow * cols, P * cols)]
            return apv.rearrange("(p c) -> p c", c=cols)[:, ds(c0, cn)]
    raise KeyError(key)


def split_into(cx, pool, src_ap, prec, tag, Tc, rows=P, bufs=1):
    nc = cx.nc
    hi = pool.tile([rows, Tc], _dt(prec), tag=f"{tag}h", bufs=bufs,
                   name=f"{tag}h_{cx.uid()}")
    nc.vector.tensor_copy(hi[:], src_ap)
    if prec != 'hilo':
        return hi, None
    tmp = pool.tile([rows, Tc], f32, tag="spt", bufs=2, name=f"spt_{cx.uid()}")
    nc.vector.tensor_sub(tmp[:], src_ap, hi[:].bitcast(f32))
    lo = pool.tile([rows, Tc], f32r, tag=f"{tag}l", bufs=bufs,
                   name=f"{tag}l_{cx.uid()}")
    nc.vector.tensor_copy(lo[:], tmp[:])
    return hi, lo


def rmsnorm(cx, pool, x_tiles, g_row, T, prec, tag):
    nc = cx.nc
    sq = pool.tile([P, T], f32, tag="nsq", bufs=2, name=f"nsq_{cx.uid()}")
    ssum = cx.psC.tile([1, T], f32, tag="mis1", name=f"nss_{cx.uid()}")
    for ko in range(KD):
        nc.vector.tensor_mul(sq[:], x_tiles[ko][:], x_tiles[ko][:])
        nc.tensor.matmul(ssum[:], cx.ones_col[:], sq[:],
                         start=(ko == 0), stop=(ko == KD - 1))
    rms = pool.tile([1, T], f32, tag="nrm", bufs=1, name=f"nrm_{cx.uid()}")
    nc.vector.tensor_scalar(rms[:], ssum[:], 1.0 / D, EPS, op0=OP.mult, op1=OP.add)
    nc.scalar.activation(rms[:], rms[:], AF.Sqrt)
    rinv = pool.tile([1, T], f32, tag="nri", bufs=1, name=f"nri_{cx.uid()}")
    nc.vector.reciprocal(rinv[:], rms[:])
    bc = cx.psC.tile([P, T], f32, tag="mis2", name=f"nbc_{cx.uid()}")
    nc.tensor.matmul(bc[:], cx.ones_row[:], rinv[:], start=True, stop=True)
    bcs = pool.tile([P, T], f32, tag="nbcs", bufs=1, name=f"nbcs_{cx.uid()}")
    nc.vector.tensor_copy(bcs[:], bc[:])
    out = []
    for ko in range(KD):
        xn = pool.tile([P, T], f32, tag="nxn", bufs=2, name=f"nxn_{cx.uid()}")
        nc.vector.tensor_mul(xn[:], x_tiles[ko][:], bcs[:])
        nc.vector.tensor_tensor(
            xn[:, None, :], xn[:, None, :],
            cx.ln_sb[:, g_row, ko, None, None].to_broadcast([P, 1, T]), OP.mult)
        out.append(split_into(cx, pool, xn[:], prec, f"{tag}{ko}", T))
    return out


def linear_fm(cx, pool, blk, wkey, xin, T, prec, Mtiles, Ktiles, out_cb):
    nc = cx.nc
    hilo = prec == 'hilo'
    for mg in range(0, Mtiles, 2):
        pts = [cx.psA.tile([P, T], f32, tag=("ps" if mi == 0 else "sc"),
                           name=f"lps{mi}_{cx.uid()}") for mi in range(2)]
        for ko in range(Ktiles):
            wh = pool.tile([P, 256], _dt(prec), tag="lwh", bufs=3,
                           name=f"lwh_{cx.uid()}")
            nc.sync.dma_start(wh[:], wview(cx, blk, wkey + '_h', ko, mg * P, 256))
            wl = None
            if hilo:
                wl = pool.tile([P, 256], f32r, tag="lwl", bufs=3,
                               name=f"lwl_{cx.uid()}")
                nc.sync.dma_start(wl[:], wview(cx, blk, wkey + '_l', ko, mg * P, 256))
            xh, xl = xin[ko]
            for mi in range(2):
                pl = _passes(prec, wh[:, ts(mi, P)],
                             wl[:, ts(mi, P)] if hilo else None,
                             xh[:], xl[:] if xl is not None else None)
                for ip, (lh_, rh_) in enumerate(pl):
                    nc.tensor.matmul(pts[mi][:], lh_, rh_,
                                     start=(ko == 0 and ip == 0),
                                     stop=(ko == Ktiles - 1 and ip == len(pl) - 1))
        for mi in range(2):
            out_cb(mg + mi, pts[mi])


def llama_block(cx, dram, x_tiles, blk, T):
    nc, tc = cx.nc, cx.tc
    prec = BLOCK_PREC[blk]
    dt = _dt(prec)
    hilo = prec == 'hilo'
    NKP = 2 if hilo else 1
    SK = T // P
    tg = f"b{blk}"

    with tc.tile_pool(name=f"bp{blk}", bufs=1) as bp:
        q_sp = [None] * KD
        kloc = dram.tile([NKP, D, T], dt, tag=f"{tg}kloc", name=f"{tg}kloc")
        vloc = dram.tile([T, H * (DH + 1)], dt, tag=f"{tg}vloc", name=f"{tg}vloc")

        with tc.tile_pool(name=f"qk{blk}", bufs=2) as sp:
            xn = rmsnorm(cx, sp, x_tiles, 2 * blk, T, prec, "xn")

            def q_cb(mo, pt):
                q_sp[mo] = split_into(cx, bp, pt[:], prec, f"qs{mo}", T)

            linear_fm(cx, sp, blk, 'wq', xn, T, prec, KD, KD, q_cb)

            def k_cb(mo, pt):
                kh, kl = split_into(cx, sp, pt[:], prec, "kk", T, bufs=2)
                nc.sync.dma_start(kloc[0, ds(mo * P, P)], kh[:])
                if kl is not None:
                    nc.sync.dma_start(kloc[1, ds(mo * P, P)], kl[:])

            linear_fm(cx, sp, blk, 'wk', xn, T, prec, KD, KD, k_cb)

            for tt in range(SK):
                vsb = sp.tile([P, H * (DH + 1)], dt, tag="vsb", bufs=2,
                              name=f"vsb_{cx.uid()}")
                nc.vector.memset(vsb[:].bitcast(f32) if dt == f32r else vsb[:], 1.0)
                for nc2 in range(D // 512):
                    pt = cx.psA.tile([P, 512], f32, tag="ps", name=f"vps_{cx.uid()}")
                    first = True
                    for ko in range(KD):
                        wvh = sp.tile([P, 512], dt, tag="wvh", bufs=3,
                                      name=f"wvh_{cx.uid()}")
                        nc.sync.dma_start(wvh[:],
                                          wview(cx, blk, 'wv_h', ko, nc2 * 512, 512))
                        wvl = None
                        if hilo:
                            wvl = sp.tile([P, 512], f32r, tag="wvl", bufs=3,
                                          name=f"wvl_{cx.uid()}")
                            nc.sync.dma_start(
                                wvl[:], wview(cx, blk, 'wv_l', ko, nc2 * 512, 512))
                        xh, xl = xn[ko]
                        pl = _passes(prec, xh[:, ts(tt, P)],
                                     xl[:, ts(tt, P)] if xl is not None else None,
                                     wvh[:], wvl[:] if hilo else None)
                        for ip, (lh_, rh_) in enumerate(pl):
                            nc.tensor.matmul(pt[:], lh_, rh_, start=first,
                                             stop=(ko == KD - 1 and ip == len(pl) - 1))
                            first = False
                    nh = 512 // DH
                    nc.vector.tensor_copy(
                        vsb[:, ds(nc2 * nh * (DH + 1), nh * (DH + 1))].rearrange(
                            "p (h e) -> p h e", e=DH + 1)[:, :, :DH],
                        pt[:].rearrange("p (h e) -> p h e", e=DH))
                nc.sync.dma_start(vloc[ds(tt * P, P)], vsb[:])

        kall = dram.tile([R, NKP, D, T], dt, tag=f"{tg}kall", name=f"{tg}kall",
                         addr_space="Shared")
        vall = dram.tile([R, T, H * (DH + 1)], dt, tag=f"{tg}vall",
                         name=f"{tg}vall", addr_space="Shared")
        nc.gpsimd.collective_compute("AllGather", OP.bypass, replica_groups=REPL,
                                     ins=[kloc[:].opt()], outs=[kall[:].opt()])
        nc.gpsimd.collective_compute("AllGather", OP.bypass, replica_groups=REPL,
                                     ins=[vloc[:].opt()], outs=[vall[:].opt()])
        kall_r = kall[:].rearrange("r n d t -> (r n d) t")
        vall_r = vall[:].rearrange("r t e -> (r t) e")

        attn_sp = [None] * KD
        with tc.tile_pool(name=f"at{blk}", bufs=2) as sp:
            for hp in range(H // 2):
                recip = sp.tile([33, T], f32, tag="rc", bufs=2, name=f"rc_{cx.uid()}")
                ovs = []
                for hpar in range(2):
                    h = 2 * hp + hpar
                    qrow = DH * hpar
                    qh_t, ql_t = q_sp[hp]
                    rh = qh_t[ds(qrow, DH)]
                    rl = ql_t[ds(qrow, DH)] if ql_t is not None else None
                    ov = cx.psB.tile([DH + 1, T], f32, tag="ov",
                                     name=f"ov_{cx.uid()}")
                    total_sk = NRANK * SK
                    isk = 0
                    for jrel in range(NRANK):
                        src = cx.srcs[jrel]
                        kbuf = sp.tile([P, NKP, T], dt, tag="kb", bufs=2,
                                       name=f"kb_{cx.uid()}")
                        for part in range(NKP):
                            nc.sync.dma_start(
                                kbuf[ds(qrow, DH), part],
                                kall_r[ds(src * (NKP * D) + part * D + h * DH, DH)])
                        for kk in range(SK):
                            sc = cx.psA.tile([P, T], f32, tag="sc",
                                             name=f"sc_{cx.uid()}")
                            kh_s = kbuf[ds(qrow, DH), 0, ts(kk, P)]
                            if hilo:
                                kl_s = kbuf[ds(qrow, DH), 1, ts(kk, P)]
                                pl = [(kh_s, rh), (kh_s, rl), (kl_s, rh)]
                            else:
                                pl = [(kh_s, rh)]
                            for ip, (lh_, rh_) in enumerate(pl):
                                nc.tensor.matmul(sc[:], lh_, rh_, start=(ip == 0),
                                                 stop=(ip == len(pl) - 1))
                            ex = sp.tile([P, T], dt, tag="ex", bufs=2,
                                         name=f"ex_{cx.uid()}")
                            if jrel == 0:
                                tmp = sp.tile([P, T], f32, tag="ext", bufs=2,
                                              name=f"ext_{cx.uid()}")
                                nc.scalar.activation(tmp[:], sc[:], AF.Exp, scale=ISQ)
                                nc.gpsimd.affine_select(
                                    ex[:], tmp[:], pattern=[[1, T]],
                                    compare_op=OP.is_ge, fill=0.0,
                                    base=-kk * P, channel_multiplier=-1)
                            else:
                                nc.scalar.activation(ex[:], sc[:], AF.Exp, scale=ISQ,
                                                     bias=cx.ab_sb[:, jrel:jrel + 1])
                            vbuf = sp.tile([P, DH + 1], dt, tag="vb", bufs=3,
                                           name=f"vb_{cx.uid()}")
                            nc.sync.dma_start(
                                vbuf[:],
                                vall_r[ds(src * T + kk * P, P),
                                       ds(h * (DH + 1), DH + 1)])
                            nc.tensor.matmul(ov[:], vbuf[:], ex[:],
                                             start=(isk == 0),
                                             stop=(isk == total_sk - 1))
                            isk += 1
                    nc.vector.reciprocal(recip[ds(32 * hpar, 1)], ov[ds(DH, 1)])
                    ovs.append(ov)
                nbc = cx.psC.tile([P, T], f32, tag="mis2", name=f"nb_{cx.uid()}")
                nc.tensor.matmul(nbc[:], cx.sel2[:], recip[:], start=True, stop=True)
                nbs = sp.tile([P, T], f32, tag="nbs", bufs=2, name=f"nbs_{cx.uid()}")
                nc.vector.tensor_copy(nbs[:], nbc[:])
                at_f = sp.tile([P, T], f32, tag="atf", bufs=2, name=f"atf_{cx.uid()}")
                nc.vector.tensor_mul(at_f[ds(0, DH)], ovs[0][ds(0, DH)],
                                     nbs[ds(0, DH)])
                nc.vector.tensor_mul(at_f[ds(DH, DH)], ovs[1][ds(0, DH)],
                                     nbs[ds(DH, DH)])
                attn_sp[hp] = split_into(cx, bp, at_f[:], prec, f"as{hp}", T)

        with tc.tile_pool(name=f"op{blk}", bufs=2) as sp:
            def o_cb(mo, pt):
                nc.vector.tensor_add(x_tiles[mo][:], x_tiles[mo][:], pt[:])

            linear_fm(cx, sp, blk, 'wo', attn_sp, T, prec, KD, KD, o_cb)

    with tc.tile_pool(name=f"ml{blk}", bufs=2) as sp:
        if True:
            xn2 = rmsnorm(cx, sp, x_tiles, 2 * blk + 1, T, prec, "xm")
            for g0 in range(0, KF, KGRP):
                gu_sp = [None] * KGRP
                for f0 in range(g0, g0 + KGRP, 2):
                    gps = [cx.psA.tile([P, T], f32, tag=t_, name=f"g{mi}_{cx.uid()}")
                           for mi, t_ in enumerate(("ps", "sc"))]
                    ups = [cx.psB.tile([P, T], f32, tag="ov", name=f"u0_{cx.uid()}"),
                           cx.psC.tile([P, T], f32, tag="mis2", name=f"u1_{cx.uid()}")]
                    for ko in range(KD):
                        wgh = sp.tile([P, 256], dt, tag="wgh", bufs=3,
                                      name=f"wgh_{cx.uid()}")
                        wuh = sp.tile([P, 256], dt, tag="wuh", bufs=3,
                                      name=f"wuh_{cx.uid()}")
                        nc.sync.dma_start(wgh[:], wview(cx, blk, 'wg_h', ko, f0 * P, 256))
                        nc.sync.dma_start(wuh[:], wview(cx, blk, 'wu_h', ko, f0 * P, 256))
                        wgl = wul = None
                        if hilo:
                            wgl = sp.tile([P, 256], f32r, tag="wgl", bufs=3,
                                          name=f"wgl_{cx.uid()}")
                            wul = sp.tile([P, 256], f32r, tag="wul", bufs=3,
                                          name=f"wul_{cx.uid()}")
                            nc.sync.dma_start(
                                wgl[:], wview(cx, blk, 'wg_l', ko, f0 * P, 256))
                            nc.sync.dma_start(
                                wul[:], wview(cx, blk, 'wu_l', ko, f0 * P, 256))
                        xh, xl = xn2[ko]
                        for mi in range(2):
                            plg = _passes(prec, wgh[:, ts(mi, P)],
                                          wgl[:, ts(mi, P)] if hilo else None,
                                          xh[:], xl[:] if xl is not None else None)
                            for ip, (lh_, rh_) in enumerate(plg):
                                nc.tensor.matmul(gps[mi][:], lh_, rh_,
                                                 start=(ko == 0 and ip == 0),
                                                 stop=(ko == KD - 1 and
                                                       ip == len(plg) - 1))
                            plu = _passes(prec, wuh[:, ts(mi, P)],
                                          wul[:, ts(mi, P)] if hilo else None,
                                          xh[:], xl[:] if xl is not None else None)
                            for ip, (lh_, rh_) in enumerate(plu):
                                nc.tensor.matmul(ups[mi][:], lh_, rh_,
                                                 start=(ko == 0 and ip == 0),
                                                 stop=(ko == KD - 1 and
                                                       ip == len(plu) - 1))
                    for mi in range(2):
                        fo = f0 + mi
                        gs = sp.tile([P, T], f32, tag="gss", bufs=2,
                                     name=f"gss_{cx.uid()}")
                        nc.scalar.activation(gs[:], gps[mi][:], AF.Silu)
                        gu_f = sp.tile([P, T], f32, tag="guf", bufs=2,
                                       name=f"guf_{cx.uid()}")
                        nc.vector.tensor_mul(gu_f[:], gs[:], ups[mi][:])
                        gu_sp[fo - g0] = split_into(cx, sp, gu_f[:], prec,
                                                    f"gu{fo - g0}", T)
                for mg in range(0, KD, 2):
                    pts = [cx.psA.tile([P, T], f32, tag=t_, name=f"d{mi}_{cx.uid()}")
                           for mi, t_ in enumerate(("ps", "sc"))]
                    for k2 in range(KGRP):
                        wdh = sp.tile([P, 256], dt, tag="wdh", bufs=3,
                                      name=f"wdh_{cx.uid()}")
                        nc.sync.dma_start(
                            wdh[:], wview(cx, blk, 'wd_h', g0 + k2, mg * P, 256))
                        wdl = None
                        if hilo:
                            wdl = sp.tile([P, 256], f32r, tag="wdl", bufs=3,
                                          name=f"wdl_{cx.uid()}")
                            nc.sync.dma_start(
                                wdl[:], wview(cx, blk, 'wd_l', g0 + k2, mg * P, 256))
                        gh, gl = gu_sp[k2]
                        for mi in range(2):
                            pl = _passes(prec, wdh[:, ts(mi, P)],
                                         wdl[:, ts(mi, P)] if hilo else None,
                                         gh[:], gl[:] if gl is not None else None)
                            for ip, (lh_, rh_) in enumerate(pl):
                                nc.tensor.matmul(pts[mi][:], lh_, rh_,
                                                 start=(k2 == 0 and ip == 0),
                                                 stop=(k2 == KGRP - 1 and
                                                       ip == len(pl) - 1))
                    for mi in range(2):
                        nc.vector.tensor_add(x_tiles[mg + mi][:],
                                             x_tiles[mg + mi][:], pts[mi][:])


def dve_matvec(cx, pool, x_tiles, rw_row, T):
    nc = cx.nc
    acc = pool.tile([P, T], f32, tag="mvac", bufs=1, name=f"mvac_{cx.uid()}")
    tmp = pool.tile([P, T], f32, tag="mvtp", bufs=1, name=f"mvtp_{cx.uid()}")
    for ko in range(KD):
        dst = acc if ko == 0 else tmp
        nc.vector.tensor_tensor(
            dst[:, None, :], x_tiles[ko][:, None, :],
            cx.rw_sb[:, rw_row, ko, None, None].to_broadcast([P, 1, T]), OP.mult)
        if ko > 0:
            nc.vector.tensor_add(acc[:], acc[:], tmp[:])
    pt = cx.psC.tile([1, T], f32, tag="mis1", name=f"mv_{cx.uid()}")
    nc.tensor.matmul(pt[:], cx.ones_col[:], acc[:], start=True, stop=True)
    lg = pool.tile([1, T], f32, tag="mvlg", bufs=1, name=f"mvlg_{cx.uid()}")
    nc.vector.tensor_copy(lg[:], pt[:])
    return lg


def bisect_mask(cx, pool, lall_flat, Sb, ktarget):
    nc = cx.nc
    nb = Sb // P
    lg = pool.tile([P, B, nb], f32, tag="bilg", bufs=1, name=f"bilg_{cx.uid()}")
    for bb in range(B):
        nc.sync.dma_start(lg[:, bb],
                          lall_flat[ds(bb * Sb, Sb)].rearrange("(p c) -> p c", c=nb))
    lo = pool.tile([P, B, nb], f32, tag="bilo", bufs=1, name=f"bilo_{cx.uid()}")
    hi = pool.tile([P, B, nb], f32, tag="bihi", bufs=1, name=f"bihi_{cx.uid()}")
    nc.vector.memset(lo[:], -16.0)
    nc.vector.memset(hi[:], 16.0)
    mid = pool.tile([P, B, nb], f32, tag="bimd", bufs=1, name=f"bimd_{cx.uid()}")
    cmp = pool.tile([P, B, nb], f32, tag="bicp", bufs=1, name=f"bicp_{cx.uid()}")
    red = pool.tile([P, B, 1], f32, tag="bird", bufs=1, name=f"bird_{cx.uid()}")
    cnt_sb = pool.tile([1, B], f32, tag="bict", bufs=1, name=f"bict_{cx.uid()}")
    pred = pool.tile([P, B], f32, tag="bipd", bufs=1, name=f"bipd_{cx.uid()}")
    dlt = pool.tile([P, B, nb], f32, tag="bidl", bufs=1, name=f"bidl_{cx.uid()}")
    for _ in range(BISECT_ITERS):
        nc.vector.tensor_add(mid[:], lo[:], hi[:])
        nc.vector.tensor_scalar_mul(mid[:], mid[:], 0.5)
        nc.vector.tensor_tensor(cmp[:], lg[:], mid[:], OP.is_gt)
        nc.vector.tensor_reduce(red[:], cmp[:], axis=mybir.AxisListType.X, op=OP.add)
        cnt = cx.psC.tile([1, B], f32, tag="mis1", name=f"bic_{cx.uid()}")
        nc.tensor.matmul(cnt[:], cx.ones_col[:], red[:, :, 0], start=True, stop=True)
        nc.vector.tensor_copy(cnt_sb[:], cnt[:])
        cbc = cx.psC.tile([P, B], f32, tag="mis2", name=f"bib_{cx.uid()}")
        nc.tensor.matmul(cbc[:], cx.ones_row[:], cnt_sb[:], start=True, stop=True)
        nc.vector.tensor_scalar(pred[:], cbc[:], float(ktarget), None, op0=OP.is_ge)
        # lo += pred * (mid - lo); hi += (1 - pred) * (mid - hi)
        nc.vector.tensor_sub(dlt[:], mid[:], lo[:])
        nc.vector.tensor_tensor(dlt[:], dlt[:],
                                pred[:, :, None].to_broadcast([P, B, nb]), OP.mult)
        nc.vector.tensor_add(lo[:], lo[:], dlt[:])
        nc.vector.tensor_scalar(pred[:], cbc[:], float(ktarget), None, op0=OP.is_lt)
        nc.vector.tensor_sub(dlt[:], mid[:], hi[:])
        nc.vector.tensor_tensor(dlt[:], dlt[:],
                                pred[:, :, None].to_broadcast([P, B, nb]), OP.mult)
        nc.vector.tensor_add(hi[:], hi[:], dlt[:])
    mask = pool.tile([P, B, nb], f32, tag="bimk", bufs=1, name=f"bimk_{cx.uid()}")
    nc.vector.tensor_tensor(mask[:], lg[:], lo[:], OP.is_gt)
    return mask


def cumsum_pos(cx, pool, dram, mask, Sb, ksel, tag):
    nc = cx.nc
    nb = Sb // P
    a = mask
    s, pp = 1, 0
    while s < nb:
        bt = pool.tile([P, B, nb], f32, tag=f"cs{pp % 2}", bufs=1,
                       name=f"cs_{cx.uid()}")
        nc.vector.tensor_copy(bt[:, :, :s], a[:, :, :s])
        nc.vector.tensor_add(bt[:, :, s:], a[:, :, s:], a[:, :, :nb - s])
        a = bt
        s *= 2
        pp += 1
    tot = pool.tile([P, B], f32, tag="cstt", bufs=1, name=f"cstt_{cx.uid()}")
    nc.vector.tensor_copy(tot[:], a[:, :, nb - 1])
    ppf = cx.psC.tile([P, B], f32, tag="mis2", name=f"csp_{cx.uid()}")
    nc.tensor.matmul(ppf[:], cx.triu[:], tot[:], start=True, stop=True)
    cs = pool.tile([P, B, nb], f32, tag="cscs", bufs=1, name=f"cscs_{cx.uid()}")
    nc.vector.tensor_tensor(cs[:], a[:], ppf[:, :, None].to_broadcast([P, B, nb]),
                            OP.add)
    csd = dram.tile([B, Sb], f32, tag=f"{tag}csd", name=f"{tag}csd")
    nc.sync.dma_start(csd[:].rearrange("b (p c) -> p b c", p=P), cs[:])
    posd = dram.tile([B * ksel, 1], f32, tag=f"{tag}posd", name=f"{tag}posd")
    for bb in range(B):
        csrow = pool.tile([1, Sb], f32, tag="cscr", bufs=1, name=f"cscr_{cx.uid()}")
        nc.sync.dma_start(csrow[:], csd[bb, None, :])
        cbc = pool.tile([P, Sb], f32, tag="cscb", bufs=1, name=f"cscb_{cx.uid()}")
        for ch in range(0, Sb, 512):
            w = min(512, Sb - ch)
            pt = cx.psC.tile([P, 512], f32, tag="mis2", name=f"csq_{cx.uid()}")
            nc.tensor.matmul(pt[:, :w], cx.ones_row[:], csrow[:, ds(ch, w)],
                             start=True, stop=True)
            nc.vector.tensor_copy(cbc[:, ds(ch, w)], pt[:, :w])
        for rt in range(ksel // P):
            rcol = pool.tile([P, 1], f32, tag="csrc", bufs=2, name=f"csrc_{cx.uid()}")
            nc.vector.tensor_scalar_add(rcol[:], cx.iota_f[:], float(rt * P))
            cmp = pool.tile([P, Sb], f32, tag="cscm", bufs=2, name=f"cscm_{cx.uid()}")
            nc.vector.tensor_tensor(cmp[:], cbc[:], rcol[:].to_broadcast([P, Sb]),
                                    OP.is_le)
            red = pool.tile([P, 1], f32, tag="csrd", bufs=2, name=f"csrd_{cx.uid()}")
            nc.vector.tensor_reduce(red[:], cmp[:], axis=mybir.AxisListType.X,
                                    op=OP.add)
            nc.sync.dma_start(posd[ds(bb * ksel + rt * P, P)], red[:])
    return posd


def to_tok_dram(cx, pool, dtile, x_tiles, T):
    nc = cx.nc
    for tt in range(T // P):
        asm = pool.tile([P, D], f32, tag="tkas", bufs=2, name=f"tkas_{cx.uid()}")
        for ko in range(KD):
            tr = cx.psC.tile([P, P], f32, tag="mis2", name=f"tktr_{cx.uid()}")
            nc.tensor.transpose(tr[:], x_tiles[ko][:, ts(tt, P)], cx.ident[:])
            nc.vector.tensor_copy(asm[:, ts(ko, P)], tr[:])
        nc.sync.dma_start(dtile[ds(tt * P, P)], asm[:])


def gather_sel(cx, pool, res, src_flat, posd, T, boff_col, rtag):
    nc = cx.nc
    myoff = cx.pid * T
    xt = [res.tile([P, T], f32, tag=f"{rtag}{ko}", name=f"{rtag}{ko}")
          for ko in range(KD)]
    for u in range(T // P):
        pv = pool.tile([P, 1], f32, tag="gspv", bufs=2, name=f"gspv_{cx.uid()}")
        nc.sync.dma_start(pv[:], posd[ds(myoff + u * P, P)])
        nc.vector.tensor_scalar(pv[:], pv[:], boff_col, None, op0=OP.add)
        pi = pool.tile([P, 1], i32, tag="gspi", bufs=2, name=f"gspi_{cx.uid()}")
        nc.vector.tensor_copy(pi[:], pv[:])
        g = pool.tile([P, D], f32, tag="gsg", bufs=2, name=f"gsg_{cx.uid()}")
        nc.gpsimd.indirect_dma_start(
            out=g[:], out_offset=None, in_=src_flat,
            in_offset=bass.IndirectOffsetOnAxis(ap=pi[:, :1], axis=0))
        for ko in range(KD):
            tr = cx.psC.tile([P, P], f32, tag="mis2", name=f"gstr_{cx.uid()}")
            nc.tensor.transpose(tr[:], g[:, ts(ko, P)], cx.ident[:])
            nc.vector.tensor_copy(xt[ko][:, ts(u, P)], tr[:])
    return xt


def topw_bcast(cx, pool, sel_in, rw_row, T):
    nc = cx.nc
    lgs = dve_matvec(cx, pool, sel_in, rw_row, T)
    tw = pool.tile([1, T], f32, tag="twr", bufs=1, name=f"twr_{cx.uid()}")
    nc.scalar.activation(tw[:], lgs[:], AF.Sigmoid)
    nc.vector.tensor_scalar_mul(tw[:], tw[:], ALPHA)
    pt = cx.psC.tile([P, T], f32, tag="mis2", name=f"twp_{cx.uid()}")
    nc.tensor.matmul(pt[:], cx.ones_row[:], tw[:], start=True, stop=True)
    twb = pool.tile([P, T], f32, tag="twb", bufs=1, name=f"twb_{cx.uid()}")
    nc.vector.tensor_copy(twb[:], pt[:])
    return twb


def build_program(stages=4, dbg=False):
    nc = bacc.Bacc("TRN2", target_bir_lowering=False)
    cx = CX()
    cx.nc = nc
    cx._u = 0

    def uid():
        cx._u += 1
        return cx._u
    cx.uid = uid

    innames = ["h0T", "ln", "rw", "abias", "fvec", "sel2c"]
    h0T = nc.declare_dram_parameter("h0T", [D, T0], f32, isOutput=False)
    lnp = nc.declare_dram_parameter("ln", [13, D], f32, isOutput=False)
    rwp = nc.declare_dram_parameter("rw", [2, D], f32, isOutput=False)
    abp = nc.declare_dram_parameter("abias", [NRANK, P], f32, isOutput=False)
    fvp = nc.declare_dram_parameter("fvec", [P, 4], f32, isOutput=False)
    s2p = nc.declare_dram_parameter("sel2c", [33, P], f32, isOutput=False)
    nblk = 6 if stages >= 3 else (3 if stages >= 2 else 1)
    wparams = {}
    for blk in range(nblk):
        items, shard = PACK_META[blk]
        npdt = f16 if BLOCK_PREC[blk] == 'f16' else f32
        wparams[blk] = nc.declare_dram_parameter(f"wpack{blk}", [1, shard], npdt,
                                                 isOutput=False)
        innames.append(f"wpack{blk}")
    out = embT = None
    if stages >= 4:
        embT = nc.declare_dram_parameter("embT", [D, VS], f32r, isOutput=False)
        out = nc.declare_dram_parameter("out", [B * S, VS], f32, isOutput=True)
        innames.append("embT")
    dbg_o = {}

    def dbg_out(nm, shp):
        dbg_o[nm] = nc.declare_dram_parameter(nm, shp, f32, isOutput=True)
        return dbg_o[nm]

    with tile.TileContext(nc) as tc:
        cx.tc = tc
        with (
            tc.tile_pool(name="const", bufs=1) as cst,
            tc.tile_pool(name="res", bufs=1) as res,
            tc.tile_pool(name="psA", bufs=2, space="PSUM") as psA,
            tc.tile_pool(name="psB", bufs=2, space="PSUM") as psB,
            tc.tile_pool(name="psC", bufs=1, space="PSUM") as psC,
            tc.tile_pool(name="dram", bufs=1, space="DRAM") as dram,
        ):
            cx.psA, cx.psB, cx.psC = psA, psB, psC

            cx.ones_col = cst.tile([P, 1], f32, name="ones_col")
            nc.vector.memset(cx.ones_col[:], 1.0)
            cx.ones_row = cst.tile([1, P], f32, name="ones_row")
            nc.vector.memset(cx.ones_row[:], 1.0)
            cx.sel2 = cst.tile([33, P], f32, name="sel2")
            nc.sync.dma_start(cx.sel2[:], s2p.ap())
            cx.ident = cst.tile([P, P], f32, name="ident")
            make_identity(nc, cx.ident[:])
            onespp = cst.tile([P, P], f32, name="onespp")
            nc.vector.memset(onespp[:], 1.0)
            cx.triu = cst.tile([P, P], f32, name="triu")
            nc.gpsimd.affine_select(cx.triu[:], onespp[:], pattern=[[1, P]],
                                    compare_op=OP.is_ge, fill=0.0, base=-1,
                                    channel_multiplier=-1)
            iota_i = cst.tile([P, 1], i32, name="iota_i")
            nc.gpsimd.iota(iota_i[:], pattern=[[0, 1]], base=0, channel_multiplier=1)
            cx.iota_f = cst.tile([P, 1], f32, name="iota_f")
            nc.vector.tensor_copy(cx.iota_f[:], iota_i[:])
            cx.ln_sb = cst.tile([P, 13, KD], f32, name="ln_sb")
            nc.sync.dma_start(cx.ln_sb[:],
                              lnp.ap().rearrange("r (ko p) -> p r ko", p=P))
            cx.rw_sb = cst.tile([P, 2, KD], f32, name="rw_sb")
            nc.sync.dma_start(cx.rw_sb[:],
                              rwp.ap().rearrange("r (ko p) -> p r ko", p=P))
            cx.ab_sb = cst.tile([P, NRANK], f32, name="ab_sb")
            nc.sync.dma_start(cx.ab_sb[:], abp.ap().rearrange("j p -> p j"))
            cx.fv_sb = cst.tile([P, 4], f32, name="fv_sb")
            nc.sync.dma_start(cx.fv_sb[:], fvp.ap())

            pid = nc.sync.partition_id()
            cx.pid = pid
            qreg = pid % NRANK
            base = pid - qreg
            cx.srcs = [smax(pid - j, base) for j in range(NRANK)]

            cx.wpacks = {}
            for blk in range(nblk):
                items, shard = PACK_META[blk]
                pdt = f16 if BLOCK_PREC[blk] == 'f16' else f32r
                wloc = dram.tile([1, shard], pdt, tag=f"wl{blk}", name=f"wl{blk}")
                nc.sync.dma_start(wloc[:], wparams[blk].ap().bitcast(pdt))
                wgat = dram.tile([R, shard], pdt, tag=f"wg{blk}", name=f"wg{blk}",
                                 addr_space="Shared")
                nc.gpsimd.collective_compute(
                    "AllGather", OP.bypass, replica_groups=REPL,
                    ins=[wloc[:].opt()], outs=[wgat[:].opt()])
                cx.wpacks[blk] = wgat[:]

            # ---- stage 1: block 0 + recursion-0 routing
            with tc.tile_pool(name="st1", bufs=1) as st1:
                x = [st1.tile([P, T0], f32, tag=f"xa{ko}", name=f"xa{ko}")
                     for ko in range(KD)]
                h0ap = h0T.ap().rearrange("(ko p) t -> p ko t", p=P)
                for ko in range(KD):
                    nc.sync.dma_start(x[ko][:], h0ap[:, ko])
                llama_block(cx, dram, x, 0, T0)

                with tc.tile_pool(name="rt0", bufs=2) as rp:
                    lg0 = dve_matvec(cx, rp, x, 0, T0)
                    lloc = dram.tile([1, T0], f32, tag="lloc0", name="lloc0")
                    nc.sync.dma_start(lloc[:], lg0[:])
                    lall = dram.tile([R, 1, T0], f32, tag="lall0", name="lall0",
                                     addr_space="Shared")
                    nc.gpsimd.collective_compute(
                        "AllGather", OP.bypass, replica_groups=REPL,
                        ins=[lloc[:].opt()], outs=[lall[:].opt()])
                    htl = dram.tile([T0, D], f32, tag="htl", name="htl")
                    to_tok_dram(cx, rp, htl, x, T0)
                    hta = dram.tile([R, T0, D], f32, tag="hta", name="hta",
                                    addr_space="Shared")
                    nc.gpsimd.collective_compute(
                        "AllGather", OP.bypass, replica_groups=REPL,
                        ins=[htl[:].opt()], outs=[hta[:].opt()])
                    cx.hta_r = hta[:].rearrange("r t d -> (r t) d")

                    mask0 = bisect_mask(cx, rp,
                                        lall[:].rearrange("r o t -> (r o t)"),
                                        S, S // 2)
                    posd0 = cumsum_pos(cx, rp, dram, mask0, S, S // 2, "c0")
                    seli = gather_sel(cx, rp, res, cx.hta_r, posd0, T1,
                                      cx.fv_sb[:, 0:1], "sli")
                    if dbg:
                        d1 = dbg_out("dbg_h0b", [T0, D])
                        nc.sync.dma_start(d1.ap(), htl[:])
                        d2 = dbg_out("dbg_lg", [1, T0])
                        nc.sync.dma_start(d2.ap(), lloc[:])
                        d3 = dbg_out("dbg_pos", [B * S // 2, 1])
                        nc.sync.dma_start(d3.ap(), posd0[:])
                        d4 = dbg_out("dbg_selT", [D, T1])
                        d4r = d4.ap().rearrange("(ko p) t -> p ko t", p=P)
                        for ko in range(KD):
                            nc.sync.dma_start(d4r[:, ko], seli[ko][:])

            if stages >= 2:
                with tc.tile_pool(name="st2", bufs=1) as st2:
                    sel = [st2.tile([P, T1], f32, tag=f"sl{ko}", name=f"sl{ko}")
                           for ko in range(KD)]
                    for ko in range(KD):
                        nc.vector.tensor_copy(sel[ko][:], seli[ko][:])
                    llama_block(cx, dram, sel, 1, T1)
                    llama_block(cx, dram, sel, 2, T1)
                    with tc.tile_pool(name="rt1", bufs=2) as rp:
                        twb0 = topw_bcast(cx, rp, seli, 0, T1)
                        x1 = [res.tile([P, T1], f32, tag=f"x1{ko}", name=f"x1{ko}")
                              for ko in range(KD)]
                        for ko in range(KD):
                            nc.vector.tensor_mul(x1[ko][:], sel[ko][:], twb0[:])
                            nc.vector.tensor_add(x1[ko][:], x1[ko][:], seli[ko][:])
                        lg1 = dve_matvec(cx, rp, x1, 1, T1)
                        lloc1 = dram.tile([1, T1], f32, tag="lloc1", name="lloc1")
                        nc.sync.dma_start(lloc1[:], lg1[:])
                        lall1 = dram.tile([R, 1, T1], f32, tag="lall1",
                                          name="lall1", addr_space="Shared")
                        nc.gpsimd.collective_compute(
                            "AllGather", OP.bypass, replica_groups=REPL,
                            ins=[lloc1[:].opt()], outs=[lall1[:].opt()])
                        x1l = dram.tile([T1, D], f32, tag="x1l", name="x1l")
                        to_tok_dram(cx, rp, x1l, x1, T1)
                        x1a = dram.tile([R, T1, D], f32, tag="x1a", name="x1a",
                                        addr_space="Shared")
                        nc.gpsimd.collective_compute(
                            "AllGather", OP.bypass, replica_groups=REPL,
                            ins=[x1l[:].opt()], outs=[x1a[:].opt()])
                        cx.x1a_r = x1a[:].rearrange("r t d -> (r t) d")

                        mask1 = bisect_mask(cx, rp,
                                            lall1[:].rearrange("r o t -> (r o t)"),
                                            S // 2, S // 4)
                        posd1 = cumsum_pos(cx, rp, dram, mask1, S // 2, S // 4, "c1")
                        sl1i = gather_sel(cx, rp, res, cx.x1a_r, posd1, T2,
                                          cx.fv_sb[:, 1:2], "s1i")
                        if dbg:
                            d5 = dbg_out("dbg_x1", [T1, D])
                            nc.sync.dma_start(d5.ap(), x1l[:])
                            d6 = dbg_out("dbg_pos1", [B * S // 4, 1])
                            nc.sync.dma_start(d6.ap(), posd1[:])

            if stages >= 3:
                with tc.tile_pool(name="st3", bufs=1) as st3:
                    sl1 = [st3.tile([P, T2], f32, tag=f"sm{ko}", name=f"sm{ko}")
                           for ko in range(KD)]
                    for ko in range(KD):
                        nc.vector.tensor_copy(sl1[ko][:], sl1i[ko][:])
                    llama_block(cx, dram, sl1, 3, T2)
                    llama_block(cx, dram, sl1, 4, T2)
                    with tc.tile_pool(name="rt2", bufs=2) as rp:
                        twb1 = topw_bcast(cx, rp, sl1i, 1, T2)
                        z = [st3.tile([P, T2], f32, tag=f"zz{ko}", name=f"zz{ko}")
                             for ko in range(KD)]
                        for ko in range(KD):
                            nc.vector.tensor_mul(z[ko][:], sl1[ko][:], twb1[:])
                            nc.vector.tensor_add(z[ko][:], z[ko][:], sl1i[ko][:])
                        zl = dram.tile([T2, D], f32, tag="zl", name="zl")
                        to_tok_dram(cx, rp, zl, z, T2)
                        za = dram.tile([R, T2, D], f32, tag="za", name="za",
                                       addr_space="Shared")
                        nc.gpsimd.collective_compute(
                            "AllGather", OP.bypass, replica_groups=REPL,
                            ins=[zl[:].opt()], outs=[za[:].opt()])
                        za_r = za[:].rearrange("r t d -> (r t) d")

                        h2loc = dram.tile([R * T0, D], f32, tag="h2loc",
                                          name="h2loc")
                        nc.sync.dma_start(h2loc[:], cx.hta_r)
                        cx.h2_r = h2loc[:]

                        for ch in range(B * S // 2 // P):
                            bb = ch // (S // 2 // P)
                            ssb = rp.tile([P, D], f32, tag="scx", bufs=2,
                                          name=f"scx_{cx.uid()}")
                            nc.sync.dma_start(ssb[:], cx.x1a_r[ds(ch * P, P)])
                            pv = rp.tile([P, 1], f32, tag="scp", bufs=2,
                                         name=f"scp_{cx.uid()}")
                            nc.sync.dma_start(pv[:], posd0[ds(ch * P, P)])
                            nc.vector.tensor_scalar_add(pv[:], pv[:], float(bb * S))
                            pi = rp.tile([P, 1], i32, tag="sci", bufs=2,
                                         name=f"sci_{cx.uid()}")
                            nc.vector.tensor_copy(pi[:], pv[:])
                            nc.gpsimd.indirect_dma_start(
                                out=cx.h2_r, out_offset=bass.IndirectOffsetOnAxis(
                                    ap=pi[:, :1], axis=0),
                                in_=ssb[:], in_offset=None)
                        for ch in range(B * S // 4 // P):
                            bb = ch // (S // 4 // P)
                            ssb = rp.tile([P, D], f32, tag="scz", bufs=2,
                                          name=f"scz_{cx.uid()}")
                            nc.sync.dma_start(ssb[:], za_r[ds(ch * P, P)])
                            p1 = rp.tile([P, 1], f32, tag="sc1", bufs=2,
                                         name=f"sc1_{cx.uid()}")
                            nc.sync.dma_start(p1[:], posd1[ds(ch * P, P)])
                            nc.vector.tensor_scalar_add(p1[:], p1[:],
                                                        float(bb * (S // 2)))
                            p1i = rp.tile([P, 1], i32, tag="sc2", bufs=2,
                                          name=f"sc2_{cx.uid()}")
                            nc.vector.tensor_copy(p1i[:], p1[:])
                            p0 = rp.tile([P, 1], f32, tag="sc3", bufs=2,
                                         name=f"sc3_{cx.uid()}")
                            nc.gpsimd.indirect_dma_start(
                                out=p0[:], out_offset=None, in_=posd0[:],
                                in_offset=bass.IndirectOffsetOnAxis(
                                    ap=p1i[:, :1], axis=0))
                            nc.vector.tensor_scalar_add(p0[:], p0[:], float(bb * S))
                            p0i = rp.tile([P, 1], i32, tag="sc4", bufs=2,
                                          name=f"sc4_{cx.uid()}")
                            nc.vector.tensor_copy(p0i[:], p0[:])
                            nc.gpsimd.indirect_dma_start(
                                out=cx.h2_r, out_offset=bass.IndirectOffsetOnAxis(
                                    ap=p0i[:, :1], axis=0),
                                in_=ssb[:], in_offset=None)
                        if dbg:
                            d7 = dbg_out("dbg_h2", [T0, D])
                            nc.sync.dma_start(d7.ap(), cx.h2_r[ds(cx.pid * T0, T0)])

            if stages >= 4:
                with tc.tile_pool(name="st4", bufs=1) as st4:
                    x5 = [st4.tile([P, T0], f32, tag=f"x5{ko}", name=f"x5{ko}")
                          for ko in range(KD)]
                    with tc.tile_pool(name="ld5", bufs=2) as rp:
                        for tt in range(T0 // P):
                            tkb = rp.tile([P, D], f32, tag="h2t", bufs=2,
                                          name=f"h2t_{cx.uid()}")
                            nc.sync.dma_start(tkb[:],
                                              cx.h2_r[ds(cx.pid * T0 + tt * P, P)])
                            for ko in range(KD):
                                tr = cx.psC.tile([P, P], f32, tag="mis2",
                                                 name=f"h2r_{cx.uid()}")
                                nc.tensor.transpose(tr[:], tkb[:, ts(ko, P)],
                                                    cx.ident[:])
                                nc.vector.tensor_copy(x5[ko][:, ts(tt, P)], tr[:])
                    llama_block(cx, dram, x5, 5, T0)
                    hfl = dram.tile([KD, P, T0], f32r, tag="hfl", name="hfl")
                    with tc.tile_pool(name="fn5", bufs=2) as rp:
                        hfn = rmsnorm(cx, rp, x5, 12, T0, 'f32r', "hf")
                        for ko in range(KD):
                            nc.sync.dma_start(hfl[ko], hfn[ko][0][:])
                    hfa = dram.tile([R, KD, P, T0], f32r, tag="hfa", name="hfa",
                                    addr_space="Shared")
                    nc.gpsimd.collective_compute(
                        "AllGather", OP.bypass, replica_groups=REPL,
                        ins=[hfl[:].opt()], outs=[hfa[:].opt()])
                with tc.tile_pool(name="hd", bufs=1) as hd:
                    NT = VS // 500
                    for ng in range(2):
                        ets = []
                        for ni in range(NT // 2):
                            nt = ng * (NT // 2) + ni
                            et = hd.tile([P, KD, 500], f32r, tag=f"et{ni}",
                                         name=f"et{ni}_{cx.uid()}")
                            for ko in range(KD):
                                nc.sync.dma_start(
                                    et[:, ko],
                                    embT.ap()[ds(ko * P, P), ds(nt * 500, 500)])
                            ets.append(et)
                        for rr in range(R):
                            hl = []
                            for ko in range(KD):
                                t_ = hd.tile([P, T0], f32r, tag=f"hl{ko}", bufs=2,
                                             name=f"hl{ko}_{cx.uid()}")
                                nc.sync.dma_start(t_[:], hfa[rr, ko])
                                hl.append(t_)
                            for tt in range(T0 // P):
                                for ni in range(NT // 2):
                                    nt = ng * (NT // 2) + ni
                                    pt = cx.psA.tile([P, 500], f32, tag="ps",
                                                     name=f"hd_{cx.uid()}")
                                    for ko in range(KD):
                                        nc.tensor.matmul(
                                            pt[:], hl[ko][:, ts(tt, P)],
                                            ets[ni][:, ko], start=(ko == 0),
                                            stop=(ko == KD - 1))
                                    ot = hd.tile([P, 500], f32, tag="hot", bufs=3,
                                                 name=f"hot_{cx.uid()}")
                                    nc.vector.tensor_copy(ot[:], pt[:])
                                    nc.sync.dma_start(
                                        out.ap()[ds(rr * T0 + tt * P, P),
                                                 ds(nt * 500, 500)],
                                        ot[:])
    nc.finalize()
    return nc, innames, list(dbg_o)


# ----------------------------------------------------------------------- host

_CACHE = {}


def _prepare_inmaps(inputs, stages):
    input_ids = np.asarray(inputs['input_ids'])
    embed = np.asarray(inputs['embed'], dtype=np.float32)
    pos_emb = np.asarray(inputs['pos_emb'], dtype=np.float32)
    h0 = embed[input_ids] + pos_emb[None, :, :]
    ln = np.empty((13, D), np.float32)
    for i in range(6):
        ln[2 * i] = inputs['ln1'][i]
        ln[2 * i + 1] = inputs['ln2'][i]
    ln[12] = inputs['final_norm']
    rw = np.asarray(inputs['router_w'], dtype=np.float32)

    nblk = 6 if stages >= 3 else (3 if stages >= 2 else 1)
    packs = {}
    for blk in range(nblk):
        prec = BLOCK_PREC[blk]
        items, shard = PACK_META[blk]
        npdt = np.float16 if prec == 'f16' else np.float32
        per_core = [np.empty(shard, npdt) for _ in range(R)]
        for key, rows, cols, off in items:
            wn, part = key.rsplit('_', 1)
            W = np.ascontiguousarray(np.asarray(inputs[REFNAMES[wn]][blk],
                                                dtype=np.float32))
            if prec == 'hilo':
                Wh = _round11(W)
                Wm = Wh if part == 'h' else (W - Wh).astype(np.float32)
            else:
                Wm = W.astype(npdt)
            rpr = rows // R
            n = rpr * cols
            for c in range(R):
                per_core[c][off:off + n] = Wm[c * rpr:(c + 1) * rpr].reshape(-1)
        packs[blk] = per_core

    in_maps = []
    for c in range(R):
        b, q = c // NRANK, c % NRANK
        m = {}
        sl = h0[b, q * T0:(q + 1) * T0]
        m['h0T'] = np.ascontiguousarray(sl.T)
        m['ln'] = ln
        m['rw'] = rw
        ab = np.zeros((NRANK, P), np.float32)
        for j in range(NRANK):
            if j > q:
                ab[j] = NEG
        m['abias'] = ab
        m['fvec'] = np.tile(np.array([[b * S, b * (S // 2), 0, 0]], np.float32),
                            (P, 1))
        s2 = np.zeros((33, P), np.float32)
        s2[0, :DH] = 1.0
        s2[32, DH:] = 1.0
        m['sel2c'] = s2
        for blk in range(nblk):
            m[f'wpack{blk}'] = packs[blk][c][None, :]
        if stages >= 4:
            m['embT'] = np.ascontiguousarray(embed[c * VS:(c + 1) * VS].T)
        in_maps.append(m)
    return in_maps


def run(inputs, stages=4, dbg=False, trace=False):
    key = (stages, dbg)
    if key not in _CACHE:
        _CACHE[key] = build_program(stages, dbg)
    nc, innames, dbgnames = _CACHE[key]
    in_maps = _prepare_inmaps(inputs, stages)
    return run_bass_kernel_spmd(nc, in_maps, core_ids=list(range(R)), trace=trace)


def kernel(**inputs):
    res = run(inputs, stages=4, dbg=False, trace=False)
    parts = [res.results[c]['out'] for c in range(R)]
    full = np.concatenate(parts, axis=1)
    return full.reshape(B, S, V).astype(np.float32)
